# revision 36
# baseline (speedup 1.0000x reference)
"""GQA attention kernel for 8 TRN2 NeuronCores — sequence-split edition.

Problem: x[4,2048,1024], 16 Q heads / 4 KV heads, head_dim 64 (torch-Linear
style projections, softmax(QK^T/8)V, output projection + bias).

Sharding: core c handles (batch b = c//2, query-half qh = c%2): it computes
the FULL output rows for its 1024 query tokens (all 16 heads + o-proj), so
there is NO collective — each core DMAs its own [1024, 1024] f32 slab out.
K/V are computed for all 2048 keys on both cores of a pair (cheap).

The host permutes each core's token order so its own queries come first
(attention is key-order invariant), which keeps the SPMD program uniform.
Q-head order is permuted on the host so each head's 64 q-dims sit at the
same partition offset as its KV head's k-dims (QK lhsT/rhs share a base
partition): device q block j (0..7), offset o in {0,64} holds head
(kv = 2*(j//4) + o//64, g = j%4); wo^T rows are permuted identically.

Inside: q^T/k^T in [dim, token] layout so QK needs no transposes; S^T is
computed in 2-key-block [128, 1024] PSUM tiles so ONE ACT exp instruction
covers both (amortizing the ~290ns ACTIVATE pipeline overhead); V is
augmented with 64 ones columns so the AV matmul also produces the softmax
denominators on partitions 64:128; normalization = DVE reciprocal + one DVE
multiply writing hid^T bf16 (custom-DVE/divide ops fail this walrus build's
codegen). The attention inner loop is software-pipelined (AV of key-pair p
emitted after QK+exp of pair p+1) and Q-proj / o-proj bursts are interleaved
into the attention stream: the PE power throttle (HAM duty cap) triggers on
dense matmul bursts, and diluting PE duty keeps the clock at full speed —
worth ~90us. A fully gapless cross-block pipeline measured SLOWER (throttle
claws back more than the bubbles save); the per-block drain is intentional.
"""

import sys
import numpy as np
from contextlib import ExitStack

sys.path.insert(0, "/opt/trn_rl_repo")

import ml_dtypes

from concourse import bass, tile, mybir


# ---------------------------------------------------------------------------
# This walrus build encodes at most 1-2 sync waits per instruction; the stock
# TileContext tail drain packs one wait per live proc onto a single Drain and
# fails codegen ("Too many sync wait commands"). Spread the waits over SP nop
# carriers instead.
def _patched_drain_and_barrier(self, tick_clock, wait_clock):
    from concourse.vector_clock import ScopedClock, VectorClock

    nc = self.nc
    gc = tick_clock.global_clock
    n = len(gc)
    for proc in range(n):
        t = gc[proc]
        if t <= 0:
            continue
        carrier = nc.sync.nop(nofuse=True)
        req = VectorClock([t if i == proc else 0 for i in range(n)])
        wait_clock.add_sem_waits(carrier.ins, ScopedClock({None: req}))
    nc.sync.drain()
    nc.all_engine_barrier()
    assert self.sems is not None
    popped = nc._tile_sem_poison_stack.pop()
    assert popped is self._sem_poison
    nc.clear_and_free_semaphores(list(self.sems.allocated().values()))
    nc.all_engine_barrier()


tile.TileContext._drain_and_barrier = _patched_drain_and_barrier


def _split_excess_waits(nc, max_waits=1):
    """Hoist all but one sync wait per instruction onto dedicated
    EventSemaphore carriers placed immediately before it on the same engine
    (same blocking semantics, one wait per encoded instruction)."""
    n_new = 0
    for bb in nc.main_func.blocks:
        il = list(bb.instructions)
        out = []
        changed = False
        for ins in il:
            si = ins.sync_info
            if si is not None:
                w = list(si.on_wait)
                if len(w) > max_waits:
                    for extra in w[max_waits:]:
                        ev = mybir.InstEventSemaphore(
                            name=f"{ins.name}-wsp{n_new}", engine=ins.engine)
                        n_new += 1
                        ev.sync_info = type(si)(on_wait=[extra], on_update=[])
                        nc.register_instruction(ev, overwrite=True)
                        out.append(ev)
                    si.on_wait = w[:max_waits]
                    changed = True
            out.append(ins)
        if changed:
            bb.instructions = out
# ---------------------------------------------------------------------------

B, N, D = 4, 2048, 1024
DH = 64          # head dim
NQ = 1024        # queries per core
NCORES = 8
P = 128
SCALE = DH ** -0.5
BF16 = mybir.dt.bfloat16
F32 = mybir.dt.float32

NKB = N // P     # 16 key blocks of 128
NKC = D // P     # 8 contraction chunks of 128
KVD = 256        # total kv dims
VW = 512         # v chunk width per key block: 4 x [64 v | 64 ones]


def build_nc(st_bufs=2, av_bufs=2, pt_bufs=4):
    nc = bass.Bass(target_bir_lowering=False, debug=False, num_devices=NCORES)

    xt = nc.declare_dram_parameter("xt", [D, N], BF16, isOutput=False)
    wqt = nc.declare_dram_parameter("wqt", [D, D], BF16, isOutput=False)
    wkt = nc.declare_dram_parameter("wkt", [D, KVD], BF16, isOutput=False)
    wvt = nc.declare_dram_parameter("wvt", [D, KVD], BF16, isOutput=False)
    wot = nc.declare_dram_parameter("wot", [D, D], BF16, isOutput=False)
    out_p = nc.declare_dram_parameter("out_p", [NQ, D], F32, isOutput=True)

    with tile.TileContext(nc) as tc, ExitStack() as ctx:
        const = ctx.enter_context(tc.tile_pool(name="const", bufs=1))
        work = ctx.enter_context(tc.tile_pool(name="work", bufs=1))
        # one shared 4-buf PSUM pool for proj/o-proj/AV tiles (deep av
        # rotation so block N+4's wait is always satisfied) + 2x 2-bank st
        ppool = ctx.enter_context(tc.tile_pool(name="ppool", bufs=4, space="PSUM"))
        stpool = ctx.enter_context(tc.tile_pool(name="stp", bufs=st_bufs, space="PSUM"))
        ptpool = ctx.enter_context(tc.tile_pool(name="ptp", bufs=pt_bufs))
        smallp = ctx.enter_context(tc.tile_pool(name="smallp", bufs=3))
        outp = ctx.enter_context(tc.tile_pool(name="outp", bufs=4))

        # ---- load inputs (K weights + x first: K-proj unblocks earliest) --
        xt_sb = const.tile([P, NKC * N], BF16)
        wkt_sb = const.tile([P, NKC * KVD], BF16)
        wvt_sb = const.tile([P, NKC * KVD], BF16)
        wqt_sb = const.tile([P, NKC * D], BF16)
        wot_sb = const.tile([P, NKC * D], BF16)
        # v (augmented with ones columns) memset FIRST: it has no deps, so
        # the 7us DVE memset runs for free during the input-DMA wait instead
        # of head-of-line blocking the projection copies
        v_sb = work.tile([P, NKB * VW], BF16, tag="v")
        nc.vector.memset(v_sb[:], 1.0)  # ones columns survive the copies
        for kc in range(NKC):
            nc.sync.dma_start(out=wkt_sb[:, kc * KVD:(kc + 1) * KVD],
                              in_=wkt[kc * P:(kc + 1) * P, :])
        # x striped in 16 half-chunks across three DGE queues so the 4MB
        # load saturates the fabric from t=0
        for h in range(2 * NKC):
            kc, hp = h // 2, (h % 2) * 64
            eng = (nc.gpsimd, nc.scalar, nc.sync)[h % 3]
            eng.dma_start(out=xt_sb[hp:hp + 64, kc * N:(kc + 1) * N],
                          in_=xt[kc * P + hp: kc * P + hp + 64, :])
        for kc in range(NKC):
            nc.sync.dma_start(out=wvt_sb[:, kc * KVD:(kc + 1) * KVD],
                              in_=wvt[kc * P:(kc + 1) * P, :])
        for kc in range(NKC):
            nc.sync.dma_start(out=wqt_sb[:, kc * D:(kc + 1) * D],
                              in_=wqt[kc * P:(kc + 1) * P, :])
        for kc in range(NKC):
            nc.sync.dma_start(out=wot_sb[:, kc * D:(kc + 1) * D],
                              in_=wot[kc * P:(kc + 1) * P, :])

        # ---- projections -------------------------------------------------
        # k^T [256, 2048] as 2 partition-blocks (kv head kv at block kv//2,
        # partition offset (kv%2)*64)
        kt_sb = work.tile([P, 2 * N], BF16, tag="kt")
        for m2 in range(2):
            for nb in range(4):
                ps = ppool.tile([P, 512], F32, tag="proj")
                for kc in range(NKC):
                    nc.tensor.matmul(
                        ps[:],
                        lhsT=wkt_sb[:, kc * KVD + m2 * P: kc * KVD + (m2 + 1) * P],
                        rhs=xt_sb[:, kc * N + nb * 512: kc * N + (nb + 1) * 512],
                        start=(kc == 0), stop=(kc == NKC - 1),
                    )
                nc.vector.tensor_copy(kt_sb[:, m2 * N + nb * 512: m2 * N + (nb + 1) * 512], ps[:])

        # v natural [keys, vdim], augmented: per key block 4 x [64 v | 64 ones]
        # (the duplicated ones columns make the AV matmul emit the softmax
        # denominators on partitions 64:128, lane-aligned with the numerators).
        # The psum->sbuf copies split between DVE and the (idle-until-
        # attention) scalar engine so the PE's V-proj never stalls on psum
        # buffers waiting for a lone DVE copy queue; ACT's `copy` shares
        # exp's table set, so no ACT_TABLE_LOAD is triggered.
        for kb in range(NKB):
            ps = ppool.tile([P, KVD], F32, tag="proj")
            for kc in range(NKC):
                nc.tensor.matmul(
                    ps[:],
                    lhsT=xt_sb[:, kc * N + kb * P: kc * N + (kb + 1) * P],
                    rhs=wvt_sb[:, kc * KVD:(kc + 1) * KVD],
                    start=(kc == 0), stop=(kc == NKC - 1),
                )
            for kv in range(4):
                dst = v_sb[:, kb * VW + kv * P: kb * VW + kv * P + 64]
                src = ps[:, kv * 64:(kv + 1) * 64]
                if kv % 2:
                    nc.scalar.copy(dst, src)
                else:
                    nc.vector.tensor_copy(dst, src)

        # q^T [1024, 1024] in device head order, 8 partition-block tiles;
        # the projection of block j is emitted right before its first
        # attention pass (dilutes the dense PE burst that trips the power
        # throttle, and lets ACT start ~40us earlier)
        qt = []
        for j in range(8):
            qt_j = work.tile([P, NQ], BF16, tag=f"qt{j}")
            qt.append(qt_j)

        # hidden^T [1024, 1024] bf16, device head order (matches wot rows)
        hid = []
        for j in range(8):
            hid_j = work.tile([P, NQ], BF16, tag=f"hid{j}")
            hid.append(hid_j)

        def oproj_tile(src_qb, tb):
            # o-proj for one 128-token slice (bias is added on the host, so
            # the PSUM just gets copied out and DMA'd per 512-col half)
            for jh in range(2):
                ps = ppool.tile([P, 512], F32, tag="proj", name="ps")
                for ic in range(8):
                    nc.tensor.matmul(
                        ps[:],
                        lhsT=hid[ic][:, src_qb * 512 + tb * P: src_qb * 512 + (tb + 1) * P],
                        rhs=wot_sb[:, ic * D + jh * 512: ic * D + (jh + 1) * 512],
                        start=(ic == 0), stop=(ic == 7),
                    )
                ot = outp.tile([P, 512], F32, tag="osb", name="ot")
                nc.vector.tensor_copy(ot[:], ps[:])
                nc.sync.dma_start(
                    out=out_p[src_qb * 512 + tb * P: src_qb * 512 + (tb + 1) * P,
                              jh * 512:(jh + 1) * 512],
                    in_=ot[:])

        # ---- attention + streamed o-proj ---------------------------------
        # One software pipeline across ALL (qb, j, o, key-pair) steps: the AV
        # matmuls for pair p are emitted after the QK+exp of pair p+1 — even
        # across block boundaries — so the PE never drains while the ACT exp
        # of a new block fills. Q-proj / bias / o-proj bursts slot into the
        # same stream, diluting PE duty below the power-throttle trigger.
        def qk_exp(j, o, qb, kb2):
            st = stpool.tile([P, 1024], F32, tag="st", name="st")
            for u in range(2):
                kb = 2 * kb2 + u
                nc.tensor.matmul(
                    st[:, u * 512:(u + 1) * 512],
                    lhsT=kt_sb[o:o + 64, (j // 4) * N + kb * P: (j // 4) * N + (kb + 1) * P],
                    rhs=qt[j][o:o + 64, qb * 512:(qb + 1) * 512],
                    start=True, stop=True,
                )
            # one ACT pass over both key blocks amortizes the ~290ns
            # ACTIVATE pipeline overhead
            pt = ptpool.tile([P, 1024], BF16, tag="pt", name="pt")
            nc.scalar.activation(pt[:], st[:],
                                 mybir.ActivationFunctionType.Exp,
                                 scale=SCALE)
            return pt

        def retire(p):
            av, kv, kb2, pt, j, o, qb = p
            for u in range(2):
                kb = 2 * kb2 + u
                nc.tensor.matmul(
                    av[:],
                    lhsT=v_sb[:, kb * VW + kv * P: kb * VW + (kv + 1) * P],
                    rhs=pt[:, u * 512:(u + 1) * 512],
                    start=(kb == 0), stop=(kb == NKB - 1),
                )
            if kb2 == NKB // 2 - 1:  # block complete: normalize into hid
                den = smallp.tile([64, 512], F32, tag="den", name="den")
                nc.vector.reciprocal(den[:], av[64:128, :])
                nc.vector.tensor_tensor(
                    out=hid[j][o:o + 64, qb * 512:(qb + 1) * 512],
                    in0=av[0:64, :], in1=den[:],
                    op=mybir.AluOpType.mult,
                )

        for qb in range(2):  # 512-query blocks
            for j in range(8):
                if qb == 0:
                    for q2 in range(2):
                        ps = ppool.tile([P, 512], F32, tag="proj")
                        for kc in range(NKC):
                            nc.tensor.matmul(
                                ps[:],
                                lhsT=wqt_sb[:, kc * D + j * P: kc * D + (j + 1) * P],
                                rhs=xt_sb[:, kc * N + q2 * 512: kc * N + (q2 + 1) * 512],
                                start=(kc == 0), stop=(kc == NKC - 1),
                            )
                        nc.vector.tensor_copy(qt[j][:, q2 * 512:(q2 + 1) * 512], ps[:])
                for o in (0, 64):
                    kv = 2 * (j // 4) + o // 64
                    av = ppool.tile([P, 512], F32, tag="proj")
                    # pipelined within the block: AV of pair p after QK of
                    # pair p+1 (the per-block drain bubble keeps PE duty just
                    # under the power-throttle trigger; a fully gapless
                    # cross-block pipeline measured SLOWER via throttle)
                    pend = None
                    for kb2 in range(NKB // 2):
                        pt = qk_exp(j, o, qb, kb2)
                        if pend is not None:
                            retire(pend)
                        pend = (av, kv, kb2, pt, j, o, qb)
                    retire(pend)

                if qb == 1 and j < 4:
                    # o-proj of query block 0 interleaved into qb1's
                    # attention (fills PE slack instead of bursting at the
                    # qb boundary)
                    oproj_tile(0, j)
        for tb in range(4):
            oproj_tile(1, tb)

    _split_excess_waits(nc)
    return nc


def make_in_maps(x, wq, wk, wv, wo, bo):
    bf = ml_dtypes.bfloat16
    # device q block j (0..7), offset o in {0,64}: head kv=2*(j//4)+o//64,
    # g=j%4; original wq row for (kv, g, lane l) = kv*256 + g*64 + l
    dperm = np.empty(D, np.int64)
    for j in range(8):
        for o in (0, 1):
            kv = 2 * (j // 4) + o
            g = j % 4
            base = j * 128 + o * 64
            dperm[base:base + 64] = np.arange(kv * 256 + g * 64, kv * 256 + g * 64 + 64)
    wqt_h = np.ascontiguousarray(wq[dperm].T).astype(bf)   # [1024, 1024 dev dims]
    wkt_h = np.ascontiguousarray(wk.T).astype(bf)          # [1024, 256]
    wvt_h = np.ascontiguousarray(wv.T).astype(bf)
    wot_h = np.ascontiguousarray(wo.T[dperm]).astype(bf)   # [1024 dev dims, 1024]
    in_maps = []
    for c in range(NCORES):
        b, qh = c // 2, c % 2
        xb = x[b]
        if qh:
            xb = np.concatenate([xb[NQ:], xb[:NQ]], axis=0)  # own queries first
        in_maps.append({
            "xt": np.ascontiguousarray(xb.T).astype(bf),
            "wqt": wqt_h,
            "wkt": wkt_h,
            "wvt": wvt_h,
            "wot": wot_h,
        })
    return in_maps


_CACHED_NC = None


def kernel(x, wq, wk, wv, wo, bo, _trace=False, _trace_kwargs=None):
    global _CACHED_NC
    from concourse.bass_utils import run_bass_kernel_spmd

    if _CACHED_NC is None:
        _CACHED_NC = build_nc()
    nc = _CACHED_NC

    in_maps = make_in_maps(
        np.asarray(x, np.float32), np.asarray(wq, np.float32),
        np.asarray(wk, np.float32), np.asarray(wv, np.float32),
        np.asarray(wo, np.float32), np.asarray(bo, np.float32))

    res = run_bass_kernel_spmd(
        nc, in_maps, core_ids=list(range(NCORES)),
        trace=_trace, **(_trace_kwargs or {}))

    out = np.empty((B, N, D), np.float32)
    for c in range(NCORES):
        b, qh = c // 2, c % 2
        out[b, qh * NQ:(qh + 1) * NQ] = res.results[c]["out_p"]
    out += np.asarray(bo, np.float32)  # bias on host, off the device tail
    if _trace:
        kernel._last_results = res
    return out
